# revision 16
# baseline (speedup 1.0000x reference)
"""AttentionGate Trainium2 kernel.

Problem (per batch b of B=32, S=2048, H=A=1024):
    keys   = seq[b] @ Wk + bk                       [S, A]
    query  = q[b] @ Wq + bq                         [A]
    align  = tanh(keys + query)                     [S, A]
    gate   = sigmoid(align @ Wg + bg)               [S, A]
    gated  = align * gate
    scores = gated @ Ws (+ bs, irrelevant after softmax)   [S]
    w      = softmax(scores)                        [S]
    ctx    = w @ seq[b]                             [H]
    pre    = concat(ctx, q[b]) @ Wo + bo + q[b]
    fused  = layernorm(pre) * gamma + beta
Returns (fused [B,H], weights [B,S]).

Sharding: data-parallel over batch, 4 batches per core on 8 cores; weights
replicated; everything for a batch is computed on one core (no collectives).

Kernel structure per core (bf16 matmuls, fp32 PSUM accumulation):
  - keys/gate contract over H/A so the PE needs seq^T (h on partitions):
    seq tiles are loaded fp32 (SP HWDGE), cast to bf16 on the otherwise-idle
    GpSimd engine, transposed via the DMA xbar (bf16-only), and also spilled
    to a DRAM bf16 scratch for the later context pass.
  - ACT applies tanh(psum+bias)/sigmoid(psum+bias) straight from PSUM.
  - scores = sum_a gated^T * Ws on the PE, software-pipelined one subtile
    late so the PE never waits on the sigmoid->mult chain.
  - softmax on the [1, S] row with a fused accumulated exp-sum.
  - context = w @ seq via PE (lhsT = w^T chunks, rhs = bf16 scratch tiles,
    contraction over s), emitted one batch late to hide the softmax round
    trip; weight loads ride the GpSimd SWDGE queue so the SP queue serves
    the seq stream.
  - final projection uses lhsT = fused_in^T so PSUM holds pre in natural
    [b, h] layout; layernorm via bn_stats/bn_aggr.
"""

import numpy as np

B_LOC = 4          # batches per core
N_CORES = 8
S = 2048           # sequence length
H = 1024           # hidden
A = 1024           # attention hidden
P = 128            # partitions
HC = H // P        # h chunks (8)
AC = A // P        # a chunks (8)
SC = S // P        # s chunks (16)
SUB = 512          # s subtile (psum free dim)
NSUB = S // SUB    # 4
K2 = 2 * H // P    # 16 chunks of the 2H contraction in the output proj
EPS = 1e-5

_CACHE = {}


def _build_module():
    import concourse.bass as bass
    import concourse.mybir as mybir
    import concourse.tile as tile
    from concourse import bacc
    from concourse.masks import make_identity

    f32 = mybir.dt.float32
    bf16 = mybir.dt.bfloat16
    AF = mybir.ActivationFunctionType
    AX = mybir.AxisListType

    nc = bacc.Bacc()

    seq = nc.dram_tensor("seq_states", [B_LOC, S, H], f32, kind="ExternalInput")
    q = nc.dram_tensor("q_state", [B_LOC, H], f32, kind="ExternalInput")
    Wk = nc.dram_tensor("Wk", [H, A], f32, kind="ExternalInput")
    bk = nc.dram_tensor("bk", [A], f32, kind="ExternalInput")
    Wq = nc.dram_tensor("Wq", [H, A], f32, kind="ExternalInput")
    bq = nc.dram_tensor("bq", [A], f32, kind="ExternalInput")
    Wg = nc.dram_tensor("Wg", [A, A], f32, kind="ExternalInput")
    bg = nc.dram_tensor("bg", [A], f32, kind="ExternalInput")
    Ws = nc.dram_tensor("Ws", [A, 1], f32, kind="ExternalInput")
    bs = nc.dram_tensor("bs", [1], f32, kind="ExternalInput")  # noqa: F841  (softmax-invariant)
    Wo = nc.dram_tensor("Wo", [2 * H, H], f32, kind="ExternalInput")
    bo = nc.dram_tensor("bo", [H], f32, kind="ExternalInput")
    gamma = nc.dram_tensor("gamma", [H], f32, kind="ExternalInput")
    beta = nc.dram_tensor("beta", [H], f32, kind="ExternalInput")

    fused_out = nc.dram_tensor("fused", [B_LOC, H], f32, kind="ExternalOutput")
    weights_out = nc.dram_tensor("weights", [B_LOC, S], f32, kind="ExternalOutput")

    def bcast(ap, n):
        # replicate a DRAM vector across n partitions (DMA-only trick)
        return bass.AP(tensor=ap.tensor, offset=ap.offset, ap=[[0, n]] + list(ap.ap))

    with tile.TileContext(nc) as tc:
        with (
            tc.tile_pool(name="singles", bufs=1) as singles,
            tc.tile_pool(name="stage", bufs=5) as stage,
            tc.tile_pool(name="natbf", bufs=6) as natbf,
            tc.tile_pool(name="ctxbf", bufs=4) as ctxbf,
            tc.tile_pool(name="seqtp", bufs=1) as seqtp,
            tc.tile_pool(name="work", bufs=1) as work,
            tc.tile_pool(name="small", bufs=2) as small,
            tc.tile_pool(name="psum", bufs=2, space="PSUM") as psum,
            tc.tile_pool(name="psum3", bufs=3, space="PSUM") as psum3,
            tc.tile_pool(name="dram", bufs=2, space="DRAM") as dram,
            tc.tile_pool(name="dram4", bufs=4, space="DRAM") as dram4,
        ):
            # ---------------- setup (ordered for fastest first-keys) ----------
            ident_f = singles.tile([P, P], f32, tag="ident_f")
            make_identity(nc, ident_f)
            ident_b = singles.tile([P, P], bf16, tag="ident_b")
            make_identity(nc, ident_b)

            # Wk first (needed by the first keys matmul); weights ride the
            # GpSimd SWDGE queue so the SP HWDGE queue can stream seq tiles.
            Wk_bf = singles.tile([P, HC, A], bf16, tag="Wk_bf")
            for hc in range(HC):
                t = stage.tile([P, A], f32, tag="wstage")
                nc.gpsimd.dma_start(out=t, in_=Wk[hc * P : (hc + 1) * P, :])
                nc.vector.tensor_copy(out=Wk_bf[:, hc, :], in_=t)

            # q and q^T
            q_nat = singles.tile([B_LOC, H], f32, tag="q_nat")
            nc.gpsimd.dma_start(out=q_nat, in_=q[:, :])
            q_nat_bf = singles.tile([B_LOC, H], bf16, tag="q_nat_bf")
            nc.vector.tensor_copy(out=q_nat_bf, in_=q_nat)

            qT_bf = singles.tile([P, HC, B_LOC], bf16, tag="qT_bf")
            fusedT_bf = singles.tile([P, K2, B_LOC], bf16, tag="fusedT_bf")
            for hc in range(HC):
                pt = psum.tile([P, B_LOC], bf16, tag="pg")
                nc.tensor.transpose(
                    pt,
                    q_nat_bf[:, hc * P : (hc + 1) * P],
                    ident_b[:B_LOC, :B_LOC],
                )
                nc.vector.tensor_copy(out=qT_bf[:, hc, :], in_=pt)
                nc.vector.tensor_copy(out=fusedT_bf[:, HC + hc, :], in_=pt)

            # bias = q @ Wq + bk + bq (natural), then transpose to columns
            pq0 = psum.tile([B_LOC, SUB], f32, tag="pk")
            pq1 = psum.tile([B_LOC, SUB], f32, tag="pk")
            for hc in range(HC):
                t = stage.tile([P, A], f32, tag="wstage")
                nc.gpsimd.dma_start(out=t, in_=Wq[hc * P : (hc + 1) * P, :])
                wqc = natbf.tile([P, A], bf16, tag="natbf")
                nc.vector.tensor_copy(out=wqc, in_=t)
                nc.tensor.matmul(
                    pq0, qT_bf[:, hc, :], wqc[:, :SUB],
                    start=(hc == 0), stop=(hc == HC - 1),
                )
                nc.tensor.matmul(
                    pq1, qT_bf[:, hc, :], wqc[:, SUB:],
                    start=(hc == 0), stop=(hc == HC - 1),
                )
            bkq_bc = singles.tile([B_LOC, A], f32, tag="bcast4")
            nc.gpsimd.dma_start(out=bkq_bc, in_=bcast(bk[:], B_LOC))
            tmp4 = stage.tile([B_LOC, A], f32, tag="stage")
            nc.gpsimd.dma_start(out=tmp4, in_=bcast(bq[:], B_LOC))
            nc.vector.tensor_add(out=bkq_bc, in0=bkq_bc, in1=tmp4)
            qq_sb = singles.tile([B_LOC, A], f32, tag="row4")
            nc.vector.tensor_copy(out=qq_sb[:, :SUB], in_=pq0)
            nc.vector.tensor_copy(out=qq_sb[:, SUB:], in_=pq1)
            nc.vector.tensor_add(out=qq_sb, in0=qq_sb, in1=bkq_bc)

            bias_f = singles.tile([P, AC, B_LOC], f32, tag="bias_f")
            for ac in range(AC):
                ptq = psum.tile([P, B_LOC], f32, tag="pg")
                nc.tensor.transpose(
                    ptq, qq_sb[:, ac * P : (ac + 1) * P], ident_f[:B_LOC, :B_LOC]
                )
                nc.vector.tensor_copy(out=bias_f[:, ac, :], in_=ptq)

            # per-a-chunk column vectors: [8, 128] rows -> PE transpose
            def load_col(vec_ap, name, dtype):
                t8 = stage.tile([AC, P], f32, tag="t8")
                nc.gpsimd.dma_start(out=t8, in_=vec_ap)
                pt = psum.tile([P, AC], f32, tag="pg")
                nc.tensor.transpose(pt, t8, ident_f[:AC, :AC])
                col = singles.tile([P, AC], dtype, tag=name)
                nc.vector.tensor_copy(out=col, in_=pt)
                return col

            bg_col = load_col(bg.rearrange("(o p) -> o p", p=P), "bg_col", f32)
            ws_col = load_col(Ws.rearrange("(o p) x -> o (p x)", p=P), "ws_col", bf16)

            # Wg (needed ~15us in)
            Wg_bf = singles.tile([P, AC, A], bf16, tag="Wg_bf")
            for ac in range(AC):
                t = stage.tile([P, A], f32, tag="wstage")
                nc.scalar.dma_start(out=t, in_=Wg[ac * P : (ac + 1) * P, :])
                nc.scalar.copy(out=Wg_bf[:, ac, :], in_=t)

            # ---------------- per-batch phases ----------------

            def emit_scores(scores, gatedT_prev, sub_prev):
                ps = psum3.tile([1, SUB], f32, tag="ps")
                for ac in range(AC):
                    nc.tensor.matmul(
                        ps,
                        ws_col[:, ac : ac + 1],
                        gatedT_prev[:, ac, :],
                        start=(ac == 0),
                        stop=(ac == AC - 1),
                    )
                nc.vector.tensor_copy(
                    out=scores[:, sub_prev * SUB : (sub_prev + 1) * SUB], in_=ps
                )

            def emit_loads_and_subtiles(b):
                """seq stream (load fp32 -> GpSimd cast -> xbar transpose ->
                bf16 DRAM spill) and the keys/gate/gated subtile loop with
                scores pipelined one subtile late (last subtile left pending)."""
                seqT = seqtp.tile([P, HC, S], bf16, tag="seqT")
                sqbf = dram.tile([SC, P, H], bf16, tag="sqbf")
                for sc in range(SC):
                    t = stage.tile([P, H], f32, tag="stage")
                    nc.sync.dma_start(out=t, in_=seq[b, sc * P : (sc + 1) * P, :])
                    nb = natbf.tile([P, H], bf16, tag="natbf")
                    nc.vector.tensor_copy(out=nb, in_=t)
                    nc.sync.dma_start_transpose(
                        out=seqT[:, :, sc * P : (sc + 1) * P], in_=nb
                    )
                    nc.gpsimd.dma_start(out=sqbf[sc], in_=nb)

                scores = work.tile([1, S], f32, tag="scores")
                pend = None

                for sub in range(NSUB):
                    ssl = slice(sub * SUB, (sub + 1) * SUB)
                    alignT = work.tile([P, AC, SUB], bf16, tag="alignT")
                    gsig = work.tile([P, AC, SUB], bf16, tag="gsig")
                    gatedT = work.tile([P, AC, SUB], bf16, tag="gatedT")

                    for ac in range(AC):
                        pk = psum.tile([P, SUB], f32, tag="pk")
                        for hc in range(HC):
                            nc.tensor.matmul(
                                pk,
                                Wk_bf[:, hc, ac * P : (ac + 1) * P],
                                seqT[:, hc, ssl],
                                start=(hc == 0),
                                stop=(hc == HC - 1),
                            )
                        nc.scalar.activation(
                            out=alignT[:, ac, :],
                            in_=pk,
                            func=AF.Tanh,
                            bias=bias_f[:, ac, b : b + 1],
                            scale=1.0,
                        )

                    if pend is not None:
                        emit_scores(scores, *pend)

                    for cc in range(AC):
                        pg = psum.tile([P, SUB], f32, tag="pg")
                        for ac in range(AC):
                            nc.tensor.matmul(
                                pg,
                                Wg_bf[:, ac, cc * P : (cc + 1) * P],
                                alignT[:, ac, :],
                                start=(ac == 0),
                                stop=(ac == AC - 1),
                            )
                        nc.scalar.activation(
                            out=gsig[:, cc, :],
                            in_=pg,
                            func=AF.Sigmoid,
                            bias=bg_col[:, cc : cc + 1],
                            scale=1.0,
                        )

                    nc.vector.tensor_mul(out=gatedT, in0=alignT, in1=gsig)
                    pend = (gatedT, sub)

                return scores, pend, sqbf

            def emit_scores_tail(b, scores, pend):
                """last subtile's scores, softmax (in place), outputs, and the
                w bf16 spill to DRAM for the context phase."""
                emit_scores(scores, *pend)
                mx = small.tile([1, 1], f32, tag="mx")
                nc.vector.reduce_max(out=mx, in_=scores, axis=AX.X)
                nmx = small.tile([1, 1], f32, tag="nmx")
                nc.vector.tensor_scalar_mul(out=nmx, in0=mx, scalar1=-1.0)
                ssum = small.tile([1, 1], f32, tag="ssum")
                nc.scalar.activation(
                    out=scores,
                    in_=scores,
                    func=AF.Exp,
                    bias=nmx,
                    scale=1.0,
                    accum_out=ssum,
                )
                rinv = small.tile([1, 1], f32, tag="rinv")
                nc.vector.reciprocal(out=rinv, in_=ssum)
                nc.vector.tensor_scalar_mul(out=scores, in0=scores, scalar1=rinv)
                nc.sync.dma_start(out=weights_out[b : b + 1, :], in_=scores)

                wrow_bf = work.tile([1, S], bf16, tag="wrow_bf")
                nc.vector.tensor_copy(out=wrow_bf, in_=scores)
                wscr = dram.tile([1, S], bf16, tag="wscr")
                nc.sync.dma_start(out=wscr, in_=wrow_bf)
                return wscr

            def emit_ctx_mms(b, wscr, sqbf):
                """ctx = w @ seq[b] via PE on the bf16 scratch, one batch late
                so the PE never waits on batch b's softmax round trip.  Leaves
                ctx (natural row) in a DRAM scratch; the transpose into
                fusedT_bf is deferred to the final stage."""
                w16 = small.tile([SC, P], bf16, tag="w16")
                nc.sync.dma_start(
                    out=w16, in_=wscr.rearrange("x (o p) -> (x o) p", p=P)
                )
                pw = psum.tile([P, SC], bf16, tag="pg")
                nc.tensor.transpose(pw, w16, ident_b[:SC, :SC])
                wT = small.tile([P, SC], bf16, tag="wT")
                nc.scalar.copy(out=wT, in_=pw)

                ctx_bf = work.tile([1, H], bf16, tag="ctx_bf")
                pc0 = psum3.tile([1, SUB], f32, tag="ps")
                pc1 = psum3.tile([1, SUB], f32, tag="ps")
                for sc in range(SC):
                    nb = ctxbf.tile([P, H], bf16, tag="ctxbf")
                    nc.sync.dma_start(out=nb, in_=sqbf[sc])
                    nc.tensor.matmul(
                        pc0,
                        wT[:, sc : sc + 1],
                        nb[:, :SUB],
                        start=(sc == 0),
                        stop=(sc == SC - 1),
                    )
                    nc.tensor.matmul(
                        pc1,
                        wT[:, sc : sc + 1],
                        nb[:, SUB:],
                        start=(sc == 0),
                        stop=(sc == SC - 1),
                    )
                nc.vector.tensor_copy(out=ctx_bf[:, :SUB], in_=pc0)
                nc.vector.tensor_copy(out=ctx_bf[:, SUB:], in_=pc1)
                cscr = dram4.tile([1, H], bf16, tag="cscr")
                nc.sync.dma_start(out=cscr, in_=ctx_bf)
                return cscr

            # Software pipeline: ctx mms of batch b-1 run between batch b's
            # last gate matmuls and its trailing scores, hiding both the
            # softmax round trip (b-1) and the sigmoid->mult chain (b).
            prev = None
            finals = []
            for b in range(B_LOC):
                scores, pend, sqbf = emit_loads_and_subtiles(b)
                if prev is not None:
                    finals.append((prev[0], emit_ctx_mms(*prev)))
                wscr = emit_scores_tail(b, scores, pend)
                prev = (b, wscr, sqbf)
            finals.append((prev[0], emit_ctx_mms(*prev)))

            # ctx^T for all batches: DRAM row -> [HC, P] -> PE transpose
            for b, cscr in finals:
                c8 = small.tile([HC, P], bf16, tag="c8")
                nc.sync.dma_start(
                    out=c8, in_=cscr.rearrange("x (o p) -> (x o) p", p=P)
                )
                pc8 = psum.tile([P, HC], bf16, tag="pg")
                nc.tensor.transpose(pc8, c8, ident_b[:HC, :HC])
                nc.vector.tensor_copy(out=fusedT_bf[:, 0:HC, b], in_=pc8)

            # ---------------- final: Wo load, projection, layernorm ----------
            Wo_bf = singles.tile([P, K2, H], bf16, tag="Wk_bf")
            for k2 in range(K2):
                t = stage.tile([P, H], f32, tag="wstage")
                nc.gpsimd.dma_start(out=t, in_=Wo[k2 * P : (k2 + 1) * P, :])
                nc.vector.tensor_copy(out=Wo_bf[:, k2, :], in_=t)

            pre = singles.tile([B_LOC, H], f32, tag="row4")
            bc4 = singles.tile([B_LOC, H], f32, tag="bcast4")
            nc.gpsimd.dma_start(out=bc4, in_=bcast(bo[:], B_LOC))
            for half in range(2):
                po = psum.tile([B_LOC, SUB], f32, tag="pk")
                for k2 in range(K2):
                    nc.tensor.matmul(
                        po,
                        fusedT_bf[:, k2, :],
                        Wo_bf[:, k2, half * SUB : (half + 1) * SUB],
                        start=(k2 == 0),
                        stop=(k2 == K2 - 1),
                    )
                nc.vector.tensor_add(
                    out=pre[:, half * SUB : (half + 1) * SUB],
                    in0=po,
                    in1=bc4[:, half * SUB : (half + 1) * SUB],
                )
            nc.vector.tensor_add(out=pre, in0=pre, in1=q_nat)

            # layernorm over H (bn_stats free-dim limit is 512 -> 2 subgroups)
            stats = small.tile([B_LOC, 2, 6], f32, tag="stats")
            pre_g = pre.rearrange("b (g d) -> b g d", g=2)
            for g in range(2):
                nc.vector.bn_stats(out=stats[:, g, :], in_=pre_g[:, g, :])
            mv = small.tile([B_LOC, 2], f32, tag="mv")
            nc.vector.bn_aggr(out=mv, in_=stats)
            eps_t = small.tile([B_LOC, 1], f32, tag="eps_t")
            nc.vector.memset(eps_t, EPS)
            rstd = small.tile([B_LOC, 1], f32, tag="rstd")
            nc.scalar.activation(
                out=rstd, in_=mv[:, 1:2], func=AF.Sqrt, bias=eps_t, scale=1.0
            )
            nc.vector.reciprocal(out=rstd, in_=rstd)
            fin = singles.tile([B_LOC, H], f32, tag="fin")
            nc.vector.tensor_scalar(
                out=fin,
                in0=pre,
                scalar1=mv[:, 0:1],
                scalar2=rstd,
                op0=mybir.AluOpType.subtract,
                op1=mybir.AluOpType.mult,
            )
            nc.gpsimd.dma_start(out=bc4, in_=bcast(gamma[:], B_LOC))
            nc.vector.tensor_mul(out=fin, in0=fin, in1=bc4)
            nc.gpsimd.dma_start(out=bc4, in_=bcast(beta[:], B_LOC))
            nc.vector.tensor_add(out=fin, in0=fin, in1=bc4)
            nc.sync.dma_start(out=fused_out[:, :], in_=fin)

    nc.finalize()
    return nc


def kernel(**inputs):
    from concourse.bass_utils import run_bass_kernel_spmd

    arr = {k: np.ascontiguousarray(np.asarray(v, dtype=np.float32)) for k, v in inputs.items()}

    if "nc" not in _CACHE:
        _CACHE["nc"] = _build_module()
    nc = _CACHE["nc"]

    shared = {
        k: arr[k]
        for k in ("Wk", "bk", "Wq", "bq", "Wg", "bg", "Ws", "bs", "Wo", "bo", "gamma", "beta")
    }
    in_maps = []
    for c in range(N_CORES):
        m = dict(shared)
        m["seq_states"] = arr["seq_states"][c * B_LOC : (c + 1) * B_LOC]
        m["q_state"] = arr["q_state"][c * B_LOC : (c + 1) * B_LOC]
        in_maps.append(m)

    res = run_bass_kernel_spmd(nc, in_maps, core_ids=list(range(N_CORES)))
    _CACHE["last_results"] = res

    fused = np.concatenate([r["fused"] for r in res.results], axis=0)
    weights = np.concatenate([r["weights"] for r in res.results], axis=0)
    return fused, weights


# revision 17
# speedup vs baseline: 1.1860x; 1.1860x over previous
"""AttentionGate Trainium2 kernel.

Problem (per batch b of B=32, S=2048, H=A=1024):
    keys   = seq[b] @ Wk + bk                       [S, A]
    query  = q[b] @ Wq + bq                         [A]
    align  = tanh(keys + query)                     [S, A]
    gate   = sigmoid(align @ Wg + bg)               [S, A]
    gated  = align * gate
    scores = gated @ Ws (+ bs, irrelevant after softmax)   [S]
    w      = softmax(scores)                        [S]
    ctx    = w @ seq[b]                             [H]
    pre    = concat(ctx, q[b]) @ Wo + bo + q[b]
    fused  = layernorm(pre) * gamma + beta
Returns (fused [B,H], weights [B,S]).

Sharding: data-parallel over batch, 4 batches per core on 8 cores; weights
replicated; everything for a batch is computed on one core (no collectives).

Kernel structure per core (bf16 matmuls, fp32 PSUM accumulation):
  - keys/gate contract over H/A so the PE needs seq^T (h on partitions):
    seq tiles are loaded fp32 (SP HWDGE), cast to bf16 on the otherwise-idle
    GpSimd engine, transposed via the DMA xbar (bf16-only), and also spilled
    to a DRAM bf16 scratch for the later context pass.
  - ACT applies tanh(psum+bias)/sigmoid(psum+bias) straight from PSUM.
  - scores = sum_a gated^T * Ws on the PE, software-pipelined one subtile
    late so the PE never waits on the sigmoid->mult chain.
  - softmax on the [1, S] row with a fused accumulated exp-sum.
  - context = w @ seq via PE (lhsT = w^T chunks, rhs = bf16 scratch tiles,
    contraction over s), emitted one batch late to hide the softmax round
    trip; weight loads ride the GpSimd SWDGE queue so the SP queue serves
    the seq stream.
  - final projection uses lhsT = fused_in^T so PSUM holds pre in natural
    [b, h] layout; layernorm via bn_stats/bn_aggr.
"""

import numpy as np

B_LOC = 4          # batches per core
N_CORES = 8
S = 2048           # sequence length
H = 1024           # hidden
A = 1024           # attention hidden
P = 128            # partitions
HC = H // P        # h chunks (8)
AC = A // P        # a chunks (8)
SC = S // P        # s chunks (16)
SUB = 512          # s subtile (psum free dim)
NSUB = S // SUB    # 4
K2 = 2 * H // P    # 16 chunks of the 2H contraction in the output proj
EPS = 1e-5

_CACHE = {}


def _build_module():
    import concourse.bass as bass
    import concourse.mybir as mybir
    import concourse.tile as tile
    from concourse import bacc
    from concourse.masks import make_identity

    f32 = mybir.dt.float32
    bf16 = mybir.dt.bfloat16
    AF = mybir.ActivationFunctionType
    AX = mybir.AxisListType

    nc = bacc.Bacc()

    seq = nc.dram_tensor("seq_states", [B_LOC, S, H], f32, kind="ExternalInput")
    q = nc.dram_tensor("q_state", [B_LOC, H], f32, kind="ExternalInput")
    Wk = nc.dram_tensor("Wk", [H, A], f32, kind="ExternalInput")
    bk = nc.dram_tensor("bk", [A], f32, kind="ExternalInput")
    Wq = nc.dram_tensor("Wq", [H, A], f32, kind="ExternalInput")
    bq = nc.dram_tensor("bq", [A], f32, kind="ExternalInput")
    Wg = nc.dram_tensor("Wg", [A, A], f32, kind="ExternalInput")
    bg = nc.dram_tensor("bg", [A], f32, kind="ExternalInput")
    Ws = nc.dram_tensor("Ws", [A, 1], f32, kind="ExternalInput")
    bs = nc.dram_tensor("bs", [1], f32, kind="ExternalInput")  # noqa: F841  (softmax-invariant)
    Wo = nc.dram_tensor("Wo", [2 * H, H], f32, kind="ExternalInput")
    bo = nc.dram_tensor("bo", [H], f32, kind="ExternalInput")
    gamma = nc.dram_tensor("gamma", [H], f32, kind="ExternalInput")
    beta = nc.dram_tensor("beta", [H], f32, kind="ExternalInput")

    fused_out = nc.dram_tensor("fused", [B_LOC, H], f32, kind="ExternalOutput")
    weights_out = nc.dram_tensor("weights", [B_LOC, S], f32, kind="ExternalOutput")

    def bcast(ap, n):
        # replicate a DRAM vector across n partitions (DMA-only trick)
        return bass.AP(tensor=ap.tensor, offset=ap.offset, ap=[[0, n]] + list(ap.ap))

    with tile.TileContext(nc) as tc:
        with (
            tc.tile_pool(name="singles", bufs=1) as singles,
            tc.tile_pool(name="stage", bufs=5) as stage,
            tc.tile_pool(name="natbf", bufs=6) as natbf,
            tc.tile_pool(name="ctxbf", bufs=4) as ctxbf,
            tc.tile_pool(name="seqtp", bufs=4) as seqtp,
            tc.tile_pool(name="work", bufs=1) as work,
            tc.tile_pool(name="small", bufs=2) as small,
            tc.tile_pool(name="psum", bufs=2, space="PSUM") as psum,
            tc.tile_pool(name="psum3", bufs=3, space="PSUM") as psum3,
            tc.tile_pool(name="dram", bufs=2, space="DRAM") as dram,
            tc.tile_pool(name="dram4", bufs=4, space="DRAM") as dram4,
        ):
            # ---------------- setup (ordered for fastest first-keys) ----------
            ident_f = singles.tile([P, P], f32, tag="ident_f")
            make_identity(nc, ident_f)
            ident_b = singles.tile([P, P], bf16, tag="ident_b")
            make_identity(nc, ident_b)

            # Wk first (needed by the first keys matmul); weights ride the
            # GpSimd SWDGE queue so the SP HWDGE queue can stream seq tiles.
            Wk_bf = singles.tile([P, HC, A], bf16, tag="Wk_bf")
            for hc in range(HC):
                t = stage.tile([P, A], f32, tag="wstage")
                nc.gpsimd.dma_start(out=t, in_=Wk[hc * P : (hc + 1) * P, :])
                nc.vector.tensor_copy(out=Wk_bf[:, hc, :], in_=t)

            # q and q^T
            q_nat = singles.tile([B_LOC, H], f32, tag="q_nat")
            nc.gpsimd.dma_start(out=q_nat, in_=q[:, :])
            q_nat_bf = singles.tile([B_LOC, H], bf16, tag="q_nat_bf")
            nc.vector.tensor_copy(out=q_nat_bf, in_=q_nat)

            qT_bf = singles.tile([P, HC, B_LOC], bf16, tag="qT_bf")
            fusedT_bf = singles.tile([P, K2, B_LOC], bf16, tag="fusedT_bf")
            for hc in range(HC):
                pt = psum.tile([P, B_LOC], bf16, tag="pg")
                nc.tensor.transpose(
                    pt,
                    q_nat_bf[:, hc * P : (hc + 1) * P],
                    ident_b[:B_LOC, :B_LOC],
                )
                nc.vector.tensor_copy(out=qT_bf[:, hc, :], in_=pt)
                nc.vector.tensor_copy(out=fusedT_bf[:, HC + hc, :], in_=pt)

            # bias = q @ Wq + bk + bq (natural), then transpose to columns
            pq0 = psum.tile([B_LOC, SUB], f32, tag="pk")
            pq1 = psum.tile([B_LOC, SUB], f32, tag="pk")
            for hc in range(HC):
                t = stage.tile([P, A], f32, tag="wstage")
                nc.gpsimd.dma_start(out=t, in_=Wq[hc * P : (hc + 1) * P, :])
                wqc = natbf.tile([P, A], bf16, tag="natbf")
                nc.vector.tensor_copy(out=wqc, in_=t)
                nc.tensor.matmul(
                    pq0, qT_bf[:, hc, :], wqc[:, :SUB],
                    start=(hc == 0), stop=(hc == HC - 1),
                )
                nc.tensor.matmul(
                    pq1, qT_bf[:, hc, :], wqc[:, SUB:],
                    start=(hc == 0), stop=(hc == HC - 1),
                )
            bkq_bc = singles.tile([B_LOC, A], f32, tag="bcast4")
            nc.gpsimd.dma_start(out=bkq_bc, in_=bcast(bk[:], B_LOC))
            tmp4 = stage.tile([B_LOC, A], f32, tag="stage")
            nc.gpsimd.dma_start(out=tmp4, in_=bcast(bq[:], B_LOC))
            nc.vector.tensor_add(out=bkq_bc, in0=bkq_bc, in1=tmp4)
            qq_sb = singles.tile([B_LOC, A], f32, tag="row4")
            nc.vector.tensor_copy(out=qq_sb[:, :SUB], in_=pq0)
            nc.vector.tensor_copy(out=qq_sb[:, SUB:], in_=pq1)
            nc.vector.tensor_add(out=qq_sb, in0=qq_sb, in1=bkq_bc)

            bias_f = singles.tile([P, AC, B_LOC], f32, tag="bias_f")
            for ac in range(AC):
                ptq = psum.tile([P, B_LOC], f32, tag="pg")
                nc.tensor.transpose(
                    ptq, qq_sb[:, ac * P : (ac + 1) * P], ident_f[:B_LOC, :B_LOC]
                )
                nc.vector.tensor_copy(out=bias_f[:, ac, :], in_=ptq)

            # per-a-chunk column vectors: [8, 128] rows -> PE transpose
            def load_col(vec_ap, name, dtype):
                t8 = stage.tile([AC, P], f32, tag="t8")
                nc.gpsimd.dma_start(out=t8, in_=vec_ap)
                pt = psum.tile([P, AC], f32, tag="pg")
                nc.tensor.transpose(pt, t8, ident_f[:AC, :AC])
                col = singles.tile([P, AC], dtype, tag=name)
                nc.vector.tensor_copy(out=col, in_=pt)
                return col

            bg_col = load_col(bg.rearrange("(o p) -> o p", p=P), "bg_col", f32)
            ws_col = load_col(Ws.rearrange("(o p) x -> o (p x)", p=P), "ws_col", bf16)

            # Wg (needed ~15us in)
            Wg_bf = singles.tile([P, AC, A], bf16, tag="Wg_bf")
            for ac in range(AC):
                t = stage.tile([P, A], f32, tag="wstage")
                nc.scalar.dma_start(out=t, in_=Wg[ac * P : (ac + 1) * P, :])
                nc.scalar.copy(out=Wg_bf[:, ac, :], in_=t)

            # ---------------- per-batch phases ----------------

            def emit_scores(scores, gatedT_prev, sub_prev):
                ps = psum3.tile([1, SUB], f32, tag="ps")
                for ac in range(AC):
                    nc.tensor.matmul(
                        ps,
                        ws_col[:, ac : ac + 1],
                        gatedT_prev[:, ac, :],
                        start=(ac == 0),
                        stop=(ac == AC - 1),
                    )
                nc.vector.tensor_copy(
                    out=scores[:, sub_prev * SUB : (sub_prev + 1) * SUB], in_=ps
                )

            def emit_loads_and_subtiles(b):
                """seq stream (load fp32 -> GpSimd cast -> xbar transpose ->
                bf16 DRAM spill) and the keys/gate/gated subtile loop with
                scores pipelined one subtile late (last subtile left pending)."""
                sqbf = dram.tile([SC, P, H], bf16, tag="sqbf")
                scores = work.tile([1, S], f32, tag="scores")
                pend = None

                for sub in range(NSUB):
                    st = seqtp.tile([P, HC, SUB], bf16, tag="seqT")
                    for j in range(SUB // P):
                        sc = sub * (SUB // P) + j
                        t = stage.tile([P, H], f32, tag="stage")
                        nc.sync.dma_start(
                            out=t, in_=seq[b, sc * P : (sc + 1) * P, :]
                        )
                        nb = natbf.tile([P, H], bf16, tag="natbf")
                        nc.vector.tensor_copy(out=nb, in_=t)
                        nc.sync.dma_start_transpose(
                            out=st[:, :, j * P : (j + 1) * P], in_=nb
                        )
                        nc.gpsimd.dma_start(out=sqbf[sc], in_=nb)

                    alignT = work.tile([P, AC, SUB], bf16, tag="alignT")
                    gsig = work.tile([P, AC, SUB], bf16, tag="gsig")
                    gatedT = work.tile([P, AC, SUB], bf16, tag="gatedT")

                    for ac in range(AC):
                        pk = psum.tile([P, SUB], f32, tag="pk")
                        for hc in range(HC):
                            nc.tensor.matmul(
                                pk,
                                Wk_bf[:, hc, ac * P : (ac + 1) * P],
                                st[:, hc, :],
                                start=(hc == 0),
                                stop=(hc == HC - 1),
                            )
                        nc.scalar.activation(
                            out=alignT[:, ac, :],
                            in_=pk,
                            func=AF.Tanh,
                            bias=bias_f[:, ac, b : b + 1],
                            scale=1.0,
                        )

                    if pend is not None:
                        emit_scores(scores, *pend)

                    for cc in range(AC):
                        pg = psum.tile([P, SUB], f32, tag="pg")
                        for ac in range(AC):
                            nc.tensor.matmul(
                                pg,
                                Wg_bf[:, ac, cc * P : (cc + 1) * P],
                                alignT[:, ac, :],
                                start=(ac == 0),
                                stop=(ac == AC - 1),
                            )
                        nc.scalar.activation(
                            out=gsig[:, cc, :],
                            in_=pg,
                            func=AF.Sigmoid,
                            bias=bg_col[:, cc : cc + 1],
                            scale=1.0,
                        )

                    nc.vector.tensor_mul(out=gatedT, in0=alignT, in1=gsig)
                    pend = (gatedT, sub)

                return scores, pend, sqbf

            def emit_scores_tail(b, scores, pend):
                """last subtile's scores, softmax (in place), outputs, and the
                w bf16 spill to DRAM for the context phase."""
                emit_scores(scores, *pend)
                mx = small.tile([1, 1], f32, tag="mx")
                nc.vector.reduce_max(out=mx, in_=scores, axis=AX.X)
                nmx = small.tile([1, 1], f32, tag="nmx")
                nc.vector.tensor_scalar_mul(out=nmx, in0=mx, scalar1=-1.0)
                ssum = small.tile([1, 1], f32, tag="ssum")
                nc.scalar.activation(
                    out=scores,
                    in_=scores,
                    func=AF.Exp,
                    bias=nmx,
                    scale=1.0,
                    accum_out=ssum,
                )
                rinv = small.tile([1, 1], f32, tag="rinv")
                nc.vector.reciprocal(out=rinv, in_=ssum)
                nc.vector.tensor_scalar_mul(out=scores, in0=scores, scalar1=rinv)
                nc.sync.dma_start(out=weights_out[b : b + 1, :], in_=scores)

                wrow_bf = work.tile([1, S], bf16, tag="wrow_bf")
                nc.vector.tensor_copy(out=wrow_bf, in_=scores)
                wscr = dram.tile([1, S], bf16, tag="wscr")
                nc.sync.dma_start(out=wscr, in_=wrow_bf)
                return wscr

            def emit_ctx_mms(b, wscr, sqbf):
                """ctx = w @ seq[b] via PE on the bf16 scratch, one batch late
                so the PE never waits on batch b's softmax round trip.  Leaves
                ctx (natural row) in a DRAM scratch; the transpose into
                fusedT_bf is deferred to the final stage."""
                w16 = small.tile([SC, P], bf16, tag="w16")
                nc.sync.dma_start(
                    out=w16, in_=wscr.rearrange("x (o p) -> (x o) p", p=P)
                )
                pw = psum.tile([P, SC], bf16, tag="pg")
                nc.tensor.transpose(pw, w16, ident_b[:SC, :SC])
                wT = small.tile([P, SC], bf16, tag="wT")
                nc.scalar.copy(out=wT, in_=pw)

                ctx_bf = work.tile([1, H], bf16, tag="ctx_bf")
                pc0 = psum3.tile([1, SUB], f32, tag="ps")
                pc1 = psum3.tile([1, SUB], f32, tag="ps")
                for sc in range(SC):
                    nb = ctxbf.tile([P, H], bf16, tag="ctxbf")
                    nc.sync.dma_start(out=nb, in_=sqbf[sc])
                    nc.tensor.matmul(
                        pc0,
                        wT[:, sc : sc + 1],
                        nb[:, :SUB],
                        start=(sc == 0),
                        stop=(sc == SC - 1),
                    )
                    nc.tensor.matmul(
                        pc1,
                        wT[:, sc : sc + 1],
                        nb[:, SUB:],
                        start=(sc == 0),
                        stop=(sc == SC - 1),
                    )
                nc.vector.tensor_copy(out=ctx_bf[:, :SUB], in_=pc0)
                nc.vector.tensor_copy(out=ctx_bf[:, SUB:], in_=pc1)
                cscr = dram4.tile([1, H], bf16, tag="cscr")
                nc.sync.dma_start(out=cscr, in_=ctx_bf)
                return cscr

            # Software pipeline: ctx mms of batch b-1 run between batch b's
            # last gate matmuls and its trailing scores, hiding both the
            # softmax round trip (b-1) and the sigmoid->mult chain (b).
            prev = None
            finals = []
            for b in range(B_LOC):
                scores, pend, sqbf = emit_loads_and_subtiles(b)
                if prev is not None:
                    finals.append((prev[0], emit_ctx_mms(*prev)))
                wscr = emit_scores_tail(b, scores, pend)
                prev = (b, wscr, sqbf)
            finals.append((prev[0], emit_ctx_mms(*prev)))

            # ctx^T for all batches: DRAM row -> [HC, P] -> PE transpose
            for b, cscr in finals:
                c8 = small.tile([HC, P], bf16, tag="c8")
                nc.sync.dma_start(
                    out=c8, in_=cscr.rearrange("x (o p) -> (x o) p", p=P)
                )
                pc8 = psum.tile([P, HC], bf16, tag="pg")
                nc.tensor.transpose(pc8, c8, ident_b[:HC, :HC])
                nc.vector.tensor_copy(out=fusedT_bf[:, 0:HC, b], in_=pc8)

            # ---------------- final: Wo load, projection, layernorm ----------
            Wo_bf = singles.tile([P, K2, H], bf16, tag="Wk_bf")
            for k2 in range(K2):
                t = stage.tile([P, H], f32, tag="wstage")
                nc.gpsimd.dma_start(out=t, in_=Wo[k2 * P : (k2 + 1) * P, :])
                nc.vector.tensor_copy(out=Wo_bf[:, k2, :], in_=t)

            pre = singles.tile([B_LOC, H], f32, tag="row4")
            bc4 = singles.tile([B_LOC, H], f32, tag="bcast4")
            nc.gpsimd.dma_start(out=bc4, in_=bcast(bo[:], B_LOC))
            for half in range(2):
                po = psum.tile([B_LOC, SUB], f32, tag="pk")
                for k2 in range(K2):
                    nc.tensor.matmul(
                        po,
                        fusedT_bf[:, k2, :],
                        Wo_bf[:, k2, half * SUB : (half + 1) * SUB],
                        start=(k2 == 0),
                        stop=(k2 == K2 - 1),
                    )
                nc.vector.tensor_add(
                    out=pre[:, half * SUB : (half + 1) * SUB],
                    in0=po,
                    in1=bc4[:, half * SUB : (half + 1) * SUB],
                )
            nc.vector.tensor_add(out=pre, in0=pre, in1=q_nat)

            # layernorm over H (bn_stats free-dim limit is 512 -> 2 subgroups)
            stats = small.tile([B_LOC, 2, 6], f32, tag="stats")
            pre_g = pre.rearrange("b (g d) -> b g d", g=2)
            for g in range(2):
                nc.vector.bn_stats(out=stats[:, g, :], in_=pre_g[:, g, :])
            mv = small.tile([B_LOC, 2], f32, tag="mv")
            nc.vector.bn_aggr(out=mv, in_=stats)
            eps_t = small.tile([B_LOC, 1], f32, tag="eps_t")
            nc.vector.memset(eps_t, EPS)
            rstd = small.tile([B_LOC, 1], f32, tag="rstd")
            nc.scalar.activation(
                out=rstd, in_=mv[:, 1:2], func=AF.Sqrt, bias=eps_t, scale=1.0
            )
            nc.vector.reciprocal(out=rstd, in_=rstd)
            fin = singles.tile([B_LOC, H], f32, tag="fin")
            nc.vector.tensor_scalar(
                out=fin,
                in0=pre,
                scalar1=mv[:, 0:1],
                scalar2=rstd,
                op0=mybir.AluOpType.subtract,
                op1=mybir.AluOpType.mult,
            )
            nc.gpsimd.dma_start(out=bc4, in_=bcast(gamma[:], B_LOC))
            nc.vector.tensor_mul(out=fin, in0=fin, in1=bc4)
            nc.gpsimd.dma_start(out=bc4, in_=bcast(beta[:], B_LOC))
            nc.vector.tensor_add(out=fin, in0=fin, in1=bc4)
            nc.sync.dma_start(out=fused_out[:, :], in_=fin)

    nc.finalize()
    return nc


def kernel(**inputs):
    from concourse.bass_utils import run_bass_kernel_spmd

    arr = {k: np.ascontiguousarray(np.asarray(v, dtype=np.float32)) for k, v in inputs.items()}

    if "nc" not in _CACHE:
        _CACHE["nc"] = _build_module()
    nc = _CACHE["nc"]

    shared = {
        k: arr[k]
        for k in ("Wk", "bk", "Wq", "bq", "Wg", "bg", "Ws", "bs", "Wo", "bo", "gamma", "beta")
    }
    in_maps = []
    for c in range(N_CORES):
        m = dict(shared)
        m["seq_states"] = arr["seq_states"][c * B_LOC : (c + 1) * B_LOC]
        m["q_state"] = arr["q_state"][c * B_LOC : (c + 1) * B_LOC]
        in_maps.append(m)

    res = run_bass_kernel_spmd(nc, in_maps, core_ids=list(range(N_CORES)))
    _CACHE["last_results"] = res

    fused = np.concatenate([r["fused"] for r in res.results], axis=0)
    weights = np.concatenate([r["weights"] for r in res.results], axis=0)
    return fused, weights
